# revision 1
# baseline (speedup 1.0000x reference)
"""v5: consecutive-src RUN packing for the gather-bound BilinearDecoder.

scores[e] = sum_j (z[src_e] @ W)[j] * z[dst_e][j] + bias, 1M edges,
8 cores, pure edge-data-parallel.

The platform's only working gather is a per-partition indirect DMA (one
offset per partition, contiguous bytes per descriptor), and kernel time
is exactly n_gather_instructions x ~1.4 us (Q7 SWDGE issue rate). So the
host chain-matches edges into CONSECUTIVE-src runs: an edge of node n
pairs with one of n+1 (93% of edges match); pairs merge into quads
(bases b, b+2) and octs (b, b+4). One descriptor of 512 B / 1 KB / 2 KB
then feeds 2 / 4 / 8 edge slots. Chunks (4096 slots) are typed
oct/quad/pair so instruction shapes stay compile-time static; region
tails are demoted downward so only the final region pads. Unmatched
edges ride with a discarded dummy partner. The dst side stays one
256 B row per slot (a partner would need consecutive src AND dst —
essentially never). Host unsorts scores via an edge-id map.

Measured: 1.813 ms, rel err 3.4e-07 (v1 single-row baseline: 2.82 ms).
"""

import numpy as np

import concourse.mybir as mybir
from concourse import bacc
from concourse.bass import IndirectOffsetOnAxis
from concourse.bass_utils import run_bass_kernel_spmd
from concourse.masks import make_identity
from concourse.tile import TileContext

N_CORES = 8
N_NODES = 100000
DIM = 64
N_EDGES = 1000000
K_SLOTS = 32
CHUNK = 128 * K_SLOTS   # 4096 slots per chunk
PAIRS_PER_CHUNK = CHUNK // 2

F32 = mybir.dt.float32
I32 = mybir.dt.int32


def build_bass(n_oct_chunks, n_quad_chunks, n_pair_chunks):
    n_chunks = n_oct_chunks + n_quad_chunks + n_pair_chunks
    e_pad = n_chunks * CHUNK
    nc = bacc.Bacc()
    z_d = nc.declare_dram_parameter("z", [N_NODES, DIM], F32, isOutput=False)
    w_d = nc.declare_dram_parameter("W", [DIM, DIM], F32, isOutput=False)
    bias_d = nc.declare_dram_parameter("biasb", [128, 1], F32, isOutput=False)
    srcp_d = nc.declare_dram_parameter("srcp", [e_pad // 2], I32, isOutput=False)
    dst_d = nc.declare_dram_parameter("dst", [e_pad], I32, isOutput=False)
    out_d = nc.declare_dram_parameter("out", [e_pad], F32, isOutput=True)

    with TileContext(nc) as tc:
        with (
            tc.tile_pool(name="const", bufs=1) as cpool,
            tc.tile_pool(name="gather", bufs=4) as gpool,
            tc.tile_pool(name="work", bufs=3) as wpool,
            tc.tile_pool(name="ps", bufs=3, space="PSUM") as ppool,
        ):
            ident = cpool.tile([128, 128], F32)
            make_identity(nc, ident[:])
            wbd = cpool.tile([128, 128], F32)
            nc.vector.memset(wbd[:], 0.0)
            nc.sync.dma_start(out=wbd[0:64, 0:64], in_=w_d[:, :])
            nc.sync.dma_start(out=wbd[64:128, 64:128], in_=w_d[:, :])
            bias_t = cpool.tile([128, 1], F32)
            nc.sync.dma_start(out=bias_t[:], in_=bias_d[:, :])
            # idx preloads: srcp[c*K/2+j] at [p, c, j]; dst at [p, c, k]
            kp = K_SLOTS // 2
            idx_all_s = cpool.tile([128, n_chunks * kp], I32)
            nc.sync.dma_start(
                out=idx_all_s[:].rearrange("p (c j) -> p c j", c=n_chunks),
                in_=srcp_d[:].rearrange("(c p j) -> p c j", c=n_chunks, p=128),
            )
            idx_all_d = cpool.tile([128, n_chunks * K_SLOTS], I32)
            nc.sync.dma_start(
                out=idx_all_d[:].rearrange("p (c k) -> p c k", c=n_chunks),
                in_=dst_d[:].rearrange("(c p k) -> p c k", c=n_chunks, p=128),
            )

            for c in range(n_chunks):
                sl = slice(c * CHUNK, (c + 1) * CHUNK)
                a_t = gpool.tile([128, K_SLOTS * DIM], F32, tag="A")
                b_t = gpool.tile([128, K_SLOTS * DIM], F32, tag="B")
                if c < n_oct_chunks:
                    # oct chunk: 2 KB descriptor covers 8 slots
                    for j in range(kp // 4):
                        col = c * kp + 4 * j
                        nc.gpsimd.indirect_dma_start(
                            out=a_t[:, (8 * j) * DIM:(8 * j + 8) * DIM],
                            out_offset=None,
                            in_=z_d[:],
                            in_offset=IndirectOffsetOnAxis(
                                ap=idx_all_s[:, col:col + 1], axis=0
                            ),
                        )
                elif c < n_oct_chunks + n_quad_chunks:
                    # quad chunk: 1 KB descriptor covers 4 slots; only
                    # every other srcp column is used (base of each quad).
                    for j in range(kp // 2):
                        col = c * kp + 2 * j
                        nc.gpsimd.indirect_dma_start(
                            out=a_t[:, (4 * j) * DIM:(4 * j + 4) * DIM],
                            out_offset=None,
                            in_=z_d[:],
                            in_offset=IndirectOffsetOnAxis(
                                ap=idx_all_s[:, col:col + 1], axis=0
                            ),
                        )
                else:
                    # pair chunk: one 512 B descriptor per PAIR of slots
                    for j in range(kp):
                        col = c * kp + j
                        nc.gpsimd.indirect_dma_start(
                            out=a_t[:, (2 * j) * DIM:(2 * j + 2) * DIM],
                            out_offset=None,
                            in_=z_d[:],
                            in_offset=IndirectOffsetOnAxis(
                                ap=idx_all_s[:, col:col + 1], axis=0
                            ),
                        )
                # dst side: one 256 B descriptor per slot
                for j in range(K_SLOTS):
                    col = c * K_SLOTS + j
                    nc.gpsimd.indirect_dma_start(
                        out=b_t[:, j * DIM:(j + 1) * DIM],
                        out_offset=None,
                        in_=z_d[:],
                        in_offset=IndirectOffsetOnAxis(
                            ap=idx_all_d[:, col:col + 1], axis=0
                        ),
                    )
                scores = wpool.tile([128, K_SLOTS], F32, tag="scores")
                for g in range(K_SLOTS // 2):
                    fs = slice(g * 128, (g + 1) * 128)
                    tp = ppool.tile([128, 128], F32, tag="tp")
                    nc.tensor.transpose(out=tp[:], in_=a_t[:, fs], identity=ident[:])
                    at = wpool.tile([128, 128], F32, tag="at")
                    nc.scalar.copy(out=at[:], in_=tp[:])
                    cp = ppool.tile([128, 128], F32, tag="cp")
                    nc.tensor.matmul(
                        out=cp[:], lhsT=at[:], rhs=wbd[:], start=True, stop=True
                    )
                    prod = wpool.tile([128, 128], F32, tag="prod")
                    nc.vector.tensor_tensor(
                        out=prod[:], in0=cp[:], in1=b_t[:, fs],
                        op=mybir.AluOpType.mult,
                    )
                    nc.vector.reduce_sum(
                        out=scores[:, g * 2:(g + 1) * 2],
                        in_=prod[:].rearrange("p (s d) -> p s d", d=DIM),
                        axis=mybir.AxisListType.X,
                    )
                nc.vector.tensor_scalar_add(
                    out=scores[:], in0=scores[:], scalar1=bias_t[:, :1]
                )
                nc.sync.dma_start(
                    out=out_d[sl].rearrange("(p k) -> p k", p=128), in_=scores[:]
                )
    nc.compile()
    return nc


def plan_pairs(src, dst):
    """Chain-match edges of node n with edges of node n+1.

    Returns (bases, even_eid, odd_eid) arrays of pair units; eid -1 =
    dummy slot.
    """
    order = np.argsort(src, kind="stable")
    deg = np.bincount(src, minlength=N_NODES)
    starts = np.zeros(N_NODES + 1, np.int64)
    np.cumsum(deg, out=starts[1:])
    bases, ev, od = [], [], []
    rem = order[0:0]
    rem_node = -1
    for n in range(N_NODES):
        cur = order[starts[n]:starts[n + 1]]
        if rem_node == n - 1 and len(rem) and len(cur):
            m = min(len(rem), len(cur))
            bases.append(np.full(m, n - 1, np.int32))
            ev.append(rem[:m])
            od.append(cur[:m])
            rem_left = rem[m:]
            cur = cur[m:]
        else:
            rem_left = rem
        # leftovers of node n-1 that can't match -> singles (even slot)
        if len(rem_left):
            bases.append(np.full(len(rem_left), rem_node, np.int32))
            ev.append(rem_left)
            od.append(np.full(len(rem_left), -1, np.int64))
        rem = cur
        rem_node = n
    if len(rem):
        # node 99999 leftovers: put real edge on even slot; base 99998
        # keeps the odd (dummy) read in bounds only if base+1 < N, so use
        # base = N-2 with real edge on ODD slot instead.
        bases.append(np.full(len(rem), N_NODES - 2, np.int32))
        ev.append(np.full(len(rem), -1, np.int64))
        od.append(rem)
    bases = np.concatenate(bases)
    ev = np.concatenate(ev).astype(np.int64)
    od = np.concatenate(od).astype(np.int64)
    return bases, ev, od


def match_runs(unit_bases, delta):
    """Greedy-match units with bases (b, b+delta). Returns (matched_pairs,
    rest_idx) over unit indices."""
    from collections import defaultdict
    used = np.zeros(len(unit_bases), bool)
    byb = defaultdict(list)
    for pos in np.argsort(unit_bases, kind="stable"):
        byb[int(unit_bases[pos])].append(pos)
    matched = []
    for b in sorted(byb):
        cur = [p for p in byb[b] if not used[p]]
        nxt = [p for p in byb.get(b + delta, []) if not used[p]]
        m = min(len(cur), len(nxt))
        for t in range(m):
            used[cur[t]] = used[nxt[t]] = True
            matched.append((cur[t], nxt[t]))
    return matched, np.where(~used)[0]


def plan_quads(bases, ev, od):
    """Greedy-match pairs with consecutive bases (b, b+2) into quads.

    Returns (quad_sel, rest_sel): index arrays into the pair list; quads
    come as consecutive entries (first=base b, second=base b+2)."""
    order = np.argsort(bases, kind="stable")
    b_sorted = bases[order]
    used = np.zeros(len(bases), bool)
    quads = []
    # bucket pair-list positions by base
    from collections import defaultdict
    byb = defaultdict(list)
    for pos in order:
        byb[int(bases[pos])].append(pos)
    for b in sorted(byb):
        cur = [p for p in byb[b] if not used[p]]
        nxt = [p for p in byb.get(b + 2, []) if not used[p]]
        m = min(len(cur), len(nxt))
        for t in range(m):
            used[cur[t]] = used[nxt[t]] = True
            quads.append((cur[t], nxt[t]))
    rest = np.where(~used)[0]
    return quads, rest


def _make_plan(src, dst):
    bases, ev, od = plan_pairs(src, dst)
    quads, rest = plan_quads(bases, ev, od)
    # oct level: match quads whose bases differ by 4
    qbases = np.array([bases[q0] for q0, q1 in quads], np.int64)
    octs, qrest_i = match_runs(qbases, 4)
    octs_units = [(quads[a][0], quads[a][1], quads[b][0], quads[b][1])
                  for a, b in octs]
    quads = [quads[i] for i in qrest_i]
    OP_CHUNK = PAIRS_PER_CHUNK // 4  # octs per chunk
    QP_CHUNK = PAIRS_PER_CHUNK // 2  # quads per chunk
    # Keep only whole oct/quad chunks (uniform across cores); demote the
    # tails (oct -> 2 quads, quad -> 2 pairs) so only the final pair
    # region pads up. Saves whole chunks (32 dst insts each).
    n_oct_chunks = (len(octs_units) // N_CORES) // OP_CHUNK
    keep_o = n_oct_chunks * OP_CHUNK * N_CORES
    for (a0, a1, b0, b1) in octs_units[keep_o:]:
        quads.append((a0, a1))
        quads.append((b0, b1))
    octs_units = octs_units[:keep_o]
    n_quad_chunks = (len(quads) // N_CORES) // QP_CHUNK
    keep_q = n_quad_chunks * QP_CHUNK * N_CORES
    demoted = [u for (a0, a1) in quads[keep_q:] for u in (a0, a1)]
    quads = quads[:keep_q]
    if demoted:
        rest = np.concatenate([rest, np.array(demoted, np.int64)])
    no = len(octs_units)
    nq = len(quads)
    nr = len(rest)
    opc = no // N_CORES
    qpc = nq // N_CORES
    rpc = -(-nr // N_CORES)
    n_pair_chunks = -(-rpc // PAIRS_PER_CHUNK)
    n_chunks = n_oct_chunks + n_quad_chunks + n_pair_chunks
    cap = n_chunks * PAIRS_PER_CHUNK
    e_pad = cap * 2
    # rebuild per-core unit lists: quad region first (as pair entries in
    # quad order), then rest pairs; pad with dummy pairs (base 0).
    per_core_units = []
    for c in range(N_CORES):
        os_ = octs_units[c * opc:(c + 1) * opc]
        uo = [u for tup in os_ for u in tup]
        uo_idx = np.array(uo, np.int64)
        opad = n_oct_chunks * PAIRS_PER_CHUNK - len(uo_idx)
        qs = quads[c * qpc:(c + 1) * qpc]
        un = []
        for q0, q1 in qs:
            un.append(q0)
            un.append(q1)
        un_idx = np.array(un, np.int64)
        qpad = n_quad_chunks * PAIRS_PER_CHUNK - len(un_idx)
        rs = rest[c * rpc:(c + 1) * rpc]
        per_core_units.append((uo_idx, opad, un_idx, qpad, rs))
    srcp = np.zeros((N_CORES, cap), np.int32)
    dummy_oct_bases = np.tile([0, 2, 4, 6], cap)
    dummy_pair_bases = np.tile([0, 2], cap)
    dstv = np.zeros((N_CORES, cap * 2), np.int32)
    eids = np.full((N_CORES, cap * 2), -1, np.int64)
    for c in range(N_CORES):
        uo_idx, opad, un_idx, qpad, rs = per_core_units[c]
        tail = cap - len(uo_idx) - opad - len(un_idx) - qpad - len(rs)
        b = np.concatenate([bases[uo_idx], dummy_oct_bases[:opad],
                            bases[un_idx], dummy_pair_bases[:qpad],
                            bases[rs], np.zeros(tail, np.int32)])
        e = np.concatenate([ev[uo_idx], np.full(opad, -1, np.int64),
                            ev[un_idx], np.full(qpad, -1, np.int64),
                            ev[rs], np.full(tail, -1, np.int64)])
        o = np.concatenate([od[uo_idx], np.full(opad, -1, np.int64),
                            od[un_idx], np.full(qpad, -1, np.int64),
                            od[rs], np.full(tail, -1, np.int64)])
        t = np.arange(cap)
        cc = t // PAIRS_PER_CHUNK
        r = t % PAIRS_PER_CHUNK
        p = r // (K_SLOTS // 2)
        j = r % (K_SLOTS // 2)
        # IMPORTANT: in quad chunks a quad's two pairs must share a
        # partition in consecutive j — unit order within a chunk must be
        # partition-major pairs: unit u of chunk cc sits at p=u//(K/2),
        # j=u%(K/2), so consecutive units already land at (p, j), (p, j+1)
        # when j is even. Quads were appended as consecutive units and
        # regions are chunk-aligned, so alignment holds.
        srcp[c, cc * PAIRS_PER_CHUNK + p * (K_SLOTS // 2) + j] = b
        es = cc * CHUNK + p * K_SLOTS + 2 * j
        valid_e = e >= 0
        valid_o = o >= 0
        dstv[c, es[valid_e]] = dst[e[valid_e]]
        dstv[c, es[valid_o] + 1] = dst[o[valid_o]]
        eids[c, es[valid_e]] = e[valid_e]
        eids[c, es[valid_o] + 1] = o[valid_o]
    return (n_oct_chunks, n_quad_chunks, n_pair_chunks), e_pad, srcp, dstv, eids


def _make_plan_v2_unused(src, dst):
    bases, ev, od = plan_pairs(src, dst)
    n_units = len(bases)
    units_per_core = -(-n_units // N_CORES)
    n_chunks = -(-units_per_core // PAIRS_PER_CHUNK)
    cap = n_chunks * PAIRS_PER_CHUNK
    e_pad = cap * 2
    srcp = np.zeros((N_CORES, cap), np.int32)
    dstv = np.zeros((N_CORES, cap * 2), np.int32)
    eids = np.full((N_CORES, cap * 2), -1, np.int64)
    for c in range(N_CORES):
        u = slice(c * units_per_core, min((c + 1) * units_per_core, n_units))
        nb = u.stop - u.start
        b, e, o = bases[u], ev[u], od[u]
        # unit t of this core -> chunk t//PPC, partition (t%PPC)//kp? No:
        # slot layout is [p, k] with k minor: pair (p, j) of chunk cc is
        # unit cc*2048 + p*(K/2) + j. Scatter accordingly.
        t = np.arange(nb)
        cc = t // PAIRS_PER_CHUNK
        r = t % PAIRS_PER_CHUNK
        p = r // (K_SLOTS // 2)
        j = r % (K_SLOTS // 2)
        srcp[c, cc * PAIRS_PER_CHUNK + p * (K_SLOTS // 2) + j] = b
        es = cc * CHUNK + p * K_SLOTS + 2 * j      # even slot flat index
        valid_e = e >= 0
        valid_o = o >= 0
        dstv[c, es[valid_e]] = dst[e[valid_e]]
        dstv[c, es[valid_o] + 1] = dst[o[valid_o]]
        eids[c, es[valid_e]] = e[valid_e]
        eids[c, es[valid_o] + 1] = o[valid_o]
    return n_chunks, e_pad, srcp, dstv, eids


_CACHE = {}


def kernel(z, edge_index, W, bias):
    z = np.ascontiguousarray(np.asarray(z, dtype=np.float32))
    W = np.ascontiguousarray(np.asarray(W, dtype=np.float32))
    bias_f = np.float32(np.asarray(bias).reshape(-1)[0])
    ei = np.asarray(edge_index)
    src = ei[0].astype(np.int64)
    dst = ei[1].astype(np.int32)
    key, e_pad, srcp, dstv, eids = _make_plan(src, dst)
    if ("nc", key) not in _CACHE:
        _CACHE[("nc", key)] = build_bass(*key)
    nc = _CACHE[("nc", key)]
    biasb = np.full((128, 1), bias_f, dtype=np.float32)
    in_maps = [
        {"z": z, "W": W, "biasb": biasb, "srcp": srcp[c], "dst": dstv[c]}
        for c in range(N_CORES)
    ]
    res = run_bass_kernel_spmd(nc, in_maps, list(range(N_CORES)))
    out = np.empty(N_EDGES, np.float32)
    for c in range(N_CORES):
        sc = np.asarray(res.results[c]["out"]).reshape(-1)
        m = eids[c] >= 0
        out[eids[c][m]] = sc[m]
    return out


def kernel_traced(z, edge_index, W, bias):
    """Same but profiled; returns (out, exec_ns)."""
    z = np.ascontiguousarray(np.asarray(z, dtype=np.float32))
    W = np.ascontiguousarray(np.asarray(W, dtype=np.float32))
    bias_f = np.float32(np.asarray(bias).reshape(-1)[0])
    ei = np.asarray(edge_index)
    src = ei[0].astype(np.int64)
    dst = ei[1].astype(np.int32)
    key, e_pad, srcp, dstv, eids = _make_plan(src, dst)
    if ("nc", key) not in _CACHE:
        _CACHE[("nc", key)] = build_bass(*key)
    nc = _CACHE[("nc", key)]
    biasb = np.full((128, 1), bias_f, dtype=np.float32)
    in_maps = [
        {"z": z, "W": W, "biasb": biasb, "srcp": srcp[c], "dst": dstv[c]}
        for c in range(N_CORES)
    ]
    res = run_bass_kernel_spmd(nc, in_maps, list(range(N_CORES)), trace=True)
    out = np.empty(N_EDGES, np.float32)
    for c in range(N_CORES):
        sc = np.asarray(res.results[c]["out"]).reshape(-1)
        m = eids[c] >= 0
        out[eids[c][m]] = sc[m]
    return out, res.exec_time_ns

